# revision 28
# baseline (speedup 1.0000x reference)
"""Trainium2 Bass kernel for the Koopman operator nn.Module.

Per-channel tiny MLPs (4 real channels, 6 complex-conjugate pairs, H=64,
2 hidden layers) over 65536 flattened batch elements, then a block-diagonal
Koopman update.  Pure data parallel over 8 NeuronCores (8192 elements each).

Design notes (v8):
  - elements on the free dim, hidden units on partitions; channels in
    block-diagonal PAIRS: hidden matmuls are [128,128]x[128,512] f32r.
  - all MLP biases are zero (spec fill=zeros; asserted on host), so the
    relu passes are pure max(x,0).
  - the 15 relu passes alternate between DVE and ACT per pair so each
    pair's relu hides under the other four pairs' matmuls; GPSIMD (Pool)
    cannot touch PSUM on real HW, so it gets all the SBUF-side work
    (x-prep, polynomial trig, rotation combines).
  - emission is software-pipelined: tile t+1's prelude and tile t-1's
    epilogue are interleaved between tile t's MLP stages so every
    engine FIFO always has independent work queued.
  - final layer packs all 16 outputs (lambda 0-3 | mu 4-9 | omega 10-15)
    into a [16,512] PSUM accumulator that shares the hidden-ps ring slot
    rotation; the elem-major T tile shares the xT ring. 6 hidden ring
    slots + xT + stk rings = 8 PSUM banks.
  - ACT keeps ONE table set resident (exp_and_others: relu+exp+identity);
    sin/cos are degree 5/4 polynomials (|omega| < 0.5 with big margin),
    so there is no per-tile activation-table reload.
  - weights ride in 3 consolidated DMAs; per-pair weights are column
    slices of the big SBUF tiles.
"""

import numpy as np

NR, NCC, L, H = 4, 6, 2, 64
B, S, C = 32, 2048, 16
NCORES = 8
F_CORE = B * S // NCORES        # 8192 elements per core
TILE = 512                      # elements per compute tile
GROUPS = TILE // 128            # 4
NT = F_CORE // TILE             # 16

_cached_nc = None

# relu engine per [pair][stage]; DVE and ACT alternate within a stage
RELU_ASSIGN = [
    ["dve", "act", "dve"],
    ["act", "dve", "act"],
    ["dve", "act", "dve"],
    ["act", "dve", "act"],
    ["dve", "act", "act"],
]


def _build():
    import concourse.tile as tile
    from concourse import bacc, mybir
    from concourse.masks import make_identity

    f32 = mybir.dt.float32
    f32r = mybir.dt.float32r
    RELU = mybir.ActivationFunctionType.Relu
    EXP = mybir.ActivationFunctionType.Exp
    IDENT = mybir.ActivationFunctionType.Identity
    ADD = mybir.AluOpType.add
    SUB = mybir.AluOpType.subtract
    MULT = mybir.AluOpType.mult

    nc = bacc.Bacc("TRN2", target_bir_lowering=False, debug=False,
                   num_devices=NCORES)

    def relu0(name, out, in_):
        if name == "act":
            nc.scalar.activation(out, in_, RELU)
        else:
            nc.vector.tensor_scalar_max(out, in_, 0.0)

    z = nc.dram_tensor("z", [F_CORE, C], f32, kind="ExternalInput").ap()
    w0p = nc.dram_tensor("w0p", [10, 5 * 128], f32r, kind="ExternalInput").ap()
    wmp = nc.dram_tensor("wmp", [128, L * 5 * 128], f32r,
                         kind="ExternalInput").ap()
    wlp = nc.dram_tensor("wlp", [128, 5 * 16], f32r, kind="ExternalInput").ap()
    out = nc.dram_tensor("out", [F_CORE, C], f32, kind="ExternalOutput").ap()

    z_r = z.rearrange("(t g p) c -> t p g c", g=GROUPS, p=128)
    out_r = out.rearrange("(t g p) c -> t p g c", g=GROUPS, p=128)

    half = TILE // 2

    with tile.TileContext(nc) as tc:
        with (
            tc.tile_pool(name="singles", bufs=1) as singles,
            tc.tile_pool(name="io", bufs=6) as io,
            tc.tile_pool(name="acts", bufs=12) as acts,
            tc.tile_pool(name="epi", bufs=14) as epi,
            tc.tile_pool(name="pshid", bufs=6, space="PSUM") as pshid,
            tc.tile_pool(name="psxT", bufs=2, space="PSUM") as psxT,
        ):
            ident = singles.tile([128, 128], f32, tag="ident")
            make_identity(nc, ident)
            ident_r = singles.tile([128, 128], f32r, tag="ident_r")
            nc.vector.tensor_copy(ident_r, ident)

            # --- 3 consolidated weight DMAs; per-pair views are column
            # slices of the big SBUF tiles ---
            w0_all = singles.tile([10, 5 * 128], f32r, tag="w0_all")
            nc.sync.dma_start(out=w0_all, in_=w0p)
            wm_all = singles.tile([128, L * 5 * 128], f32r, tag="wm_all")
            nc.sync.dma_start(out=wm_all, in_=wmp)
            wl_all = singles.tile([128, 5 * 16], f32r, tag="wl_all")
            nc.sync.dma_start(out=wl_all, in_=wlp)

            w0_sb = [w0_all[:, j * 128:(j + 1) * 128] for j in range(5)]
            wm_sb = [[wm_all[:, (l * 5 + j) * 128:(l * 5 + j + 1) * 128]
                      for j in range(5)] for l in range(L)]
            wl_sb = [wl_all[:, j * 16:(j + 1) * 16] for j in range(5)]

            # GPSIMD (Pool) on real HW supports only plain tensor_tensor
            # (add/sub/mult) in SBUF, so the trig polynomial uses TT chains
            # against broadcast constant tiles.
            P = nc.gpsimd
            k24 = singles.tile([128, GROUPS, 6], f32, tag="k24")
            P.memset(k24, 1.0 / 24)
            kmh = singles.tile([128, GROUPS, 6], f32, tag="kmh")
            P.memset(kmh, -0.5)
            k120 = singles.tile([128, GROUPS, 6], f32, tag="k120")
            P.memset(k120, 1.0 / 120)
            km6 = singles.tile([128, GROUPS, 6], f32, tag="km6")
            P.memset(km6, -1.0 / 6)

            def emit_prelude(t):
                """DMA z, build x_nat, transpose to xT, evacuate to SBUF."""
                z_nat = io.tile([128, GROUPS, C], f32, name=f"z_nat_{t}",
                                tag="z_nat")
                nc.sync.dma_start(out=z_nat, in_=z_r[t])
                z1 = z_nat[:, :, 4:16:2]
                z2 = z_nat[:, :, 5:16:2]

                x_nat = io.tile([128, GROUPS, 10], f32r, name=f"x_nat_{t}",
                                tag="x_nat")
                nc.vector.tensor_copy(x_nat[:, :, 0:4], z_nat[:, :, 0:4])
                m1 = epi.tile([128, GROUPS, 6], f32, name=f"m1_{t}", tag="m1")
                P.tensor_tensor(m1, z1, z1, MULT)
                m2 = epi.tile([128, GROUPS, 6], f32, name=f"m2_{t}", tag="m2")
                P.tensor_tensor(m2, z2, z2, MULT)
                P.tensor_tensor(x_nat[:, :, 4:10], m1, m2, ADD)

                xT_fl = psxT.tile([128, TILE], f32r, name=f"xT_{t}", tag="xt")
                xT_ps = xT_fl[0:10, :]
                for g in range(GROUPS):
                    nc.tensor.transpose(
                        xT_ps[:, g * 128:(g + 1) * 128], x_nat[:, g, :],
                        ident_r)
                xT = acts.tile([10, TILE], f32r, name=f"xTs_{t}", tag="xT_sb")
                nc.vector.tensor_copy(xT[:, 0:half], xT_ps[:, 0:half])
                nc.scalar.activation(xT[:, half:TILE], xT_ps[:, half:TILE],
                                     IDENT)
                return z_nat, xT

            def emit_late(t, z_nat, stk_sb):
                """T transpose + evac, exp, polynomial trig, rotation."""
                z1 = z_nat[:, :, 4:16:2]
                z2 = z_nat[:, :, 5:16:2]

                T_fl = psxT.tile([128, TILE], f32r, name=f"T_{t}", tag="xt")
                for g in range(GROUPS):
                    nc.tensor.transpose(
                        T_fl[:, g * 16:(g + 1) * 16],
                        stk_sb[:, g * 128:(g + 1) * 128],
                        ident_r[0:16, 0:16])
                T_sb = epi.tile([128, GROUPS, 16], f32, name=f"Ts_{t}",
                                tag="T_sb")
                nc.vector.tensor_copy(T_sb, T_fl[:, 0:GROUPS * 16])

                lamT = T_sb[:, :, 0:4]
                muT = T_sb[:, :, 4:10]
                omT = T_sb[:, :, 10:16]

                e = epi.tile([128, GROUPS, 6], f32, name=f"e_{t}", tag="e")
                nc.scalar.activation(e, muT, EXP)

                # cos(om)*e ~ e + r*e       with r = (w2/24 - 1/2)*w2
                # sin(om)*e ~ (p*om + om)*e with p = (w2/120 - 1/6)*w2
                w2 = epi.tile([128, GROUPS, 6], f32, name=f"w2_{t}", tag="w2")
                P.tensor_tensor(w2, omT, omT, MULT)
                tc_ = epi.tile([128, GROUPS, 6], f32, name=f"tc_{t}", tag="tc")
                P.tensor_tensor(tc_, w2, k24, MULT)
                P.tensor_tensor(tc_, tc_, kmh, ADD)
                rc = epi.tile([128, GROUPS, 6], f32, name=f"rc_{t}", tag="rc")
                P.tensor_tensor(rc, tc_, w2, MULT)
                mc = epi.tile([128, GROUPS, 6], f32, name=f"mc_{t}", tag="mc")
                P.tensor_tensor(mc, rc, e, MULT)
                P.tensor_tensor(mc, mc, e, ADD)
                ts_ = epi.tile([128, GROUPS, 6], f32, name=f"ts_{t}", tag="ts")
                P.tensor_tensor(ts_, w2, k120, MULT)
                P.tensor_tensor(ts_, ts_, km6, ADD)
                rs = epi.tile([128, GROUPS, 6], f32, name=f"rs_{t}", tag="rs")
                P.tensor_tensor(rs, ts_, w2, MULT)
                sn = epi.tile([128, GROUPS, 6], f32, name=f"sn_{t}", tag="sn")
                P.tensor_tensor(sn, rs, omT, MULT)
                P.tensor_tensor(sn, sn, omT, ADD)
                ms = epi.tile([128, GROUPS, 6], f32, name=f"ms_{t}", tag="ms")
                P.tensor_tensor(ms, sn, e, MULT)

                # out_r = zr*lam; o1 = z1*mc + z2*ms; o2 = z2*mc - z1*ms
                o_nat = io.tile([128, GROUPS, C], f32, name=f"o_nat_{t}",
                                tag="o_nat")
                P.tensor_tensor(o_nat[:, :, 0:4], z_nat[:, :, 0:4], lamT,
                                MULT)
                t1 = epi.tile([128, GROUPS, 6], f32, name=f"t1_{t}", tag="t1")
                t2 = epi.tile([128, GROUPS, 6], f32, name=f"t2_{t}", tag="t2")
                P.tensor_tensor(t1, z1, mc, MULT)
                P.tensor_tensor(t2, z2, ms, MULT)
                P.tensor_tensor(o_nat[:, :, 4:16:2], t1, t2, ADD)
                t3 = epi.tile([128, GROUPS, 6], f32, name=f"t3_{t}", tag="t3")
                t4 = epi.tile([128, GROUPS, 6], f32, name=f"t4_{t}", tag="t4")
                P.tensor_tensor(t3, z2, mc, MULT)
                P.tensor_tensor(t4, z1, ms, MULT)
                P.tensor_tensor(o_nat[:, :, 5:16:2], t3, t4, SUB)

                nc.sync.dma_start(out=out_r[t], in_=o_nat)

            # --- software-pipelined main loop ---
            late_args = None      # tile t-1 epilogue inputs
            cur = emit_prelude(0)
            for t in range(NT):
                z_nat, xT = cur
                rhs = [xT] * 5
                pss = [None] * 5

                for s in range(3):
                    weights = w0_sb if s == 0 else wm_sb[s - 1]
                    for j in range(5):
                        pss[j] = pshid.tile([128, TILE], f32,
                                            name=f"ps_{t}_{s}_{j}", tag="ps")
                        nc.tensor.matmul(pss[j], weights[j], rhs[j],
                                         start=True, stop=True)
                    if s == 0 and t + 1 < NT:
                        # next tile's prelude: PE transposes + Pool x-prep
                        # fill the gap before this tile's stage-1 matmuls
                        cur = emit_prelude(t + 1)
                    if s == 1 and late_args is not None:
                        # previous tile's epilogue fills the stage-2 gap
                        emit_late(*late_args)
                        late_args = None
                    for j in range(5):
                        h = acts.tile([128, TILE], f32r,
                                      name=f"h_{t}_{s}_{j}", tag="h")
                        relu0(RELU_ASSIGN[j][s], h, pss[j])
                        rhs[j] = h

                # stk shares the pshid ring slot rotation
                stk_fl = pshid.tile([128, TILE], f32, name=f"stk_{t}",
                                    tag="ps")
                stk_ps = stk_fl[0:16, :]
                for j in range(5):
                    nc.tensor.matmul(stk_ps, wl_sb[j], rhs[j],
                                     start=(j == 0), stop=(j == 4))

                # evacuate stk to SBUF, split across DVE and ACT
                # (biases are all zero per the spec)
                stk_sb = acts.tile([16, TILE], f32r, name=f"stks_{t}",
                                   tag="stk_sb")
                nc.vector.tensor_copy(stk_sb[:, 0:half], stk_ps[:, 0:half])
                nc.scalar.activation(stk_sb[:, half:TILE],
                                     stk_ps[:, half:TILE], IDENT)

                late_args = (t, z_nat, stk_sb)

            emit_late(*late_args)

    nc.compile()
    return nc


def _pack_weights(i):
    """Pack per-channel weights into block-diagonal pair form.

    All biases must be zero (guaranteed by the problem spec, fill=zeros);
    the kernel folds that assumption into pure-relu passes.
    """
    f32 = np.float32
    for k in ("b0_r", "bm_r", "bl_r", "b0_c", "bm_c", "bl_c"):
        assert not np.any(np.asarray(i[k])), f"nonzero bias {k}"
    W0_r = np.asarray(i["W0_r"], f32)
    Wm_r = np.asarray(i["Wm_r"], f32)
    Wl_r = np.asarray(i["Wl_r"], f32)
    W0_c = np.asarray(i["W0_c"], f32)
    Wm_c = np.asarray(i["Wm_c"], f32)
    Wl_c = np.asarray(i["Wl_c"], f32)

    w0p = np.zeros((5, 10, 128), f32)
    wmp = np.zeros((L, 5, 128, 128), f32)
    wlp = np.zeros((5, 128, 16), f32)
    for j in range(5):
        if j < 2:
            a, b = 2 * j, 2 * j + 1
            W0, Wm = W0_r, Wm_r
        else:
            a, b = 2 * (j - 2), 2 * (j - 2) + 1
            W0, Wm = W0_c, Wm_c
        r0 = 2 * j if j < 2 else 4 + 2 * (j - 2)
        w0p[j, r0, 0:64] = W0[a]
        w0p[j, r0 + 1, 64:128] = W0[b]
        for l in range(L):
            wmp[l, j, 0:64, 0:64] = Wm[l, a]
            wmp[l, j, 64:128, 64:128] = Wm[l, b]
        if j < 2:
            wlp[j, 0:64, 2 * j] = Wl_r[a][:, 0]
            wlp[j, 64:128, 2 * j + 1] = Wl_r[b][:, 0]
        else:
            jc = j - 2
            wlp[j, 0:64, 4 + 2 * jc] = Wl_c[a][:, 0]        # mu_a
            wlp[j, 64:128, 5 + 2 * jc] = Wl_c[b][:, 0]      # mu_b
            wlp[j, 0:64, 10 + 2 * jc] = Wl_c[a][:, 1]       # om_a
            wlp[j, 64:128, 11 + 2 * jc] = Wl_c[b][:, 1]     # om_b

    w0_all = np.concatenate([w0p[j] for j in range(5)], axis=1)      # [10,640]
    wm_all = np.concatenate(
        [wmp[l, j] for l in range(L) for j in range(5)], axis=1)  # [128,1280]
    wl_all = np.concatenate([wlp[j] for j in range(5)], axis=1)      # [128,80]
    return {"w0p": np.ascontiguousarray(w0_all),
            "wmp": np.ascontiguousarray(wm_all),
            "wlp": np.ascontiguousarray(wl_all)}


def kernel(**inputs):
    global _cached_nc
    if _cached_nc is None:
        _cached_nc = _build()
    nc = _cached_nc

    from concourse.bass_utils import run_bass_kernel_spmd

    weights = _pack_weights(inputs)
    z = np.ascontiguousarray(np.asarray(inputs["z"], np.float32)
                             .reshape(NCORES, F_CORE, C))
    in_maps = [dict(weights, z=z[i]) for i in range(NCORES)]
    res = run_bass_kernel_spmd(nc, in_maps, core_ids=list(range(NCORES)))
    outs = [np.asarray(res.results[i]["out"]) for i in range(NCORES)]
    return np.concatenate(outs, axis=0).reshape(B, S, C)


# revision 30
# speedup vs baseline: 1.0293x; 1.0293x over previous
"""Trainium2 Bass kernel for the Koopman operator nn.Module.

Per-channel tiny MLPs (4 real channels, 6 complex-conjugate pairs, H=64,
2 hidden layers) over 65536 flattened batch elements, then a block-diagonal
Koopman update.  Pure data parallel over 8 NeuronCores (8192 elements each).

Design notes (v8):
  - elements on the free dim, hidden units on partitions; channels in
    block-diagonal PAIRS: hidden matmuls are [128,128]x[128,512] f32r.
  - all MLP biases are zero (spec fill=zeros; asserted on host), so the
    relu passes are pure max(x,0).
  - the 15 relu passes alternate between DVE and ACT per pair so each
    pair's relu hides under the other four pairs' matmuls; GPSIMD (Pool)
    cannot touch PSUM on real HW, so it gets all the SBUF-side work
    (x-prep, polynomial trig, rotation combines).
  - emission is software-pipelined: tile t+1's prelude and tile t-1's
    epilogue are interleaved between tile t's MLP stages so every
    engine FIFO always has independent work queued.
  - final layer packs all 16 outputs (lambda 0-3 | mu 4-9 | omega 10-15)
    into a [16,512] PSUM accumulator that shares the hidden-ps ring slot
    rotation; the elem-major T tile shares the xT ring. 6 hidden ring
    slots + xT + stk rings = 8 PSUM banks.
  - ACT keeps ONE table set resident (exp_and_others: relu+exp+identity);
    sin/cos are degree 5/4 polynomials (|omega| < 0.5 with big margin),
    so there is no per-tile activation-table reload.
  - weights ride in 3 consolidated DMAs; per-pair weights are column
    slices of the big SBUF tiles.
"""

import numpy as np

NR, NCC, L, H = 4, 6, 2, 64
B, S, C = 32, 2048, 16
NCORES = 8
F_CORE = B * S // NCORES        # 8192 elements per core
TILE = 512                      # elements per compute tile
GROUPS = TILE // 128            # 4
NT = F_CORE // TILE             # 16

_cached_nc = None

# relu engine per [pair][stage]; DVE and ACT alternate within a stage
RELU_ASSIGN = [
    ["dve", "act", "dve"],
    ["act", "dve", "act"],
    ["dve", "act", "dve"],
    ["act", "dve", "act"],
    ["dve", "act", "act"],
]


def _build():
    import concourse.tile as tile
    from concourse import bacc, mybir
    from concourse.masks import make_identity

    f32 = mybir.dt.float32
    f32r = mybir.dt.float32r
    RELU = mybir.ActivationFunctionType.Relu
    EXP = mybir.ActivationFunctionType.Exp
    IDENT = mybir.ActivationFunctionType.Identity
    ADD = mybir.AluOpType.add
    SUB = mybir.AluOpType.subtract
    MULT = mybir.AluOpType.mult

    nc = bacc.Bacc("TRN2", target_bir_lowering=False, debug=False,
                   num_devices=NCORES)

    def relu0(name, out, in_):
        if name == "act":
            nc.scalar.activation(out, in_, RELU)
        else:
            nc.vector.tensor_scalar_max(out, in_, 0.0)

    z = nc.dram_tensor("z", [F_CORE, C], f32, kind="ExternalInput").ap()
    w0p = nc.dram_tensor("w0p", [10, 5 * 128], f32r, kind="ExternalInput").ap()
    wmp = nc.dram_tensor("wmp", [128, L * 5 * 128], f32r,
                         kind="ExternalInput").ap()
    wlp = nc.dram_tensor("wlp", [128, 5 * 16], f32r, kind="ExternalInput").ap()
    out = nc.dram_tensor("out", [F_CORE, C], f32, kind="ExternalOutput").ap()

    z_r = z.rearrange("(t g p) c -> t p g c", g=GROUPS, p=128)
    out_r = out.rearrange("(t g p) c -> t p g c", g=GROUPS, p=128)

    half = TILE // 2

    with tile.TileContext(nc) as tc:
        with (
            tc.tile_pool(name="singles", bufs=1) as singles,
            tc.tile_pool(name="io", bufs=6) as io,
            tc.tile_pool(name="acts", bufs=12) as acts,
            tc.tile_pool(name="epi", bufs=14) as epi,
            tc.tile_pool(name="pshid", bufs=6, space="PSUM") as pshid,
            tc.tile_pool(name="psxT", bufs=2, space="PSUM") as psxT,
        ):
            ident = singles.tile([128, 128], f32, tag="ident")
            make_identity(nc, ident)
            ident_r = singles.tile([128, 128], f32r, tag="ident_r")
            nc.vector.tensor_copy(ident_r, ident)

            # --- 3 consolidated weight DMAs; per-pair views are column
            # slices of the big SBUF tiles ---
            w0_all = singles.tile([10, 5 * 128], f32r, tag="w0_all")
            nc.sync.dma_start(out=w0_all, in_=w0p)
            wm_all = singles.tile([128, L * 5 * 128], f32r, tag="wm_all")
            nc.sync.dma_start(out=wm_all, in_=wmp)
            wl_all = singles.tile([128, 5 * 16], f32r, tag="wl_all")
            nc.sync.dma_start(out=wl_all, in_=wlp)

            w0_sb = [w0_all[:, j * 128:(j + 1) * 128] for j in range(5)]
            wm_sb = [[wm_all[:, (l * 5 + j) * 128:(l * 5 + j + 1) * 128]
                      for j in range(5)] for l in range(L)]
            wl_sb = [wl_all[:, j * 16:(j + 1) * 16] for j in range(5)]

            # GPSIMD (Pool) on real HW supports only plain tensor_tensor
            # (add/sub/mult) in SBUF, so the trig polynomial uses TT chains
            # against broadcast constant tiles.
            P = nc.gpsimd
            k24 = singles.tile([128, GROUPS, 6], f32, tag="k24")
            P.memset(k24, 1.0 / 24)
            kmh = singles.tile([128, GROUPS, 6], f32, tag="kmh")
            P.memset(kmh, -0.5)
            k120 = singles.tile([128, GROUPS, 6], f32, tag="k120")
            P.memset(k120, 1.0 / 120)
            km6 = singles.tile([128, GROUPS, 6], f32, tag="km6")
            P.memset(km6, -1.0 / 6)
            # exp(mu) Taylor coefficients 1/k! (mu in [-0.6, 0.4], deg 6)
            kexp = []
            fact = 1.0
            for k in range(7):
                fact = fact * max(k, 1)
                kt = singles.tile([128, GROUPS, 6], f32, tag=f"kexp{k}")
                P.memset(kt, 1.0 / fact)
                kexp.append(kt)

            def emit_prelude(t):
                """DMA z, build x_nat, transpose to xT, evacuate to SBUF."""
                z_nat = io.tile([128, GROUPS, C], f32, name=f"z_nat_{t}",
                                tag="z_nat")
                nc.sync.dma_start(out=z_nat, in_=z_r[t])
                z1 = z_nat[:, :, 4:16:2]
                z2 = z_nat[:, :, 5:16:2]

                x_nat = io.tile([128, GROUPS, 10], f32r, name=f"x_nat_{t}",
                                tag="x_nat")
                nc.vector.tensor_copy(x_nat[:, :, 0:4], z_nat[:, :, 0:4])
                m1 = epi.tile([128, GROUPS, 6], f32, name=f"m1_{t}", tag="m1")
                P.tensor_tensor(m1, z1, z1, MULT)
                m2 = epi.tile([128, GROUPS, 6], f32, name=f"m2_{t}", tag="m2")
                P.tensor_tensor(m2, z2, z2, MULT)
                P.tensor_tensor(x_nat[:, :, 4:10], m1, m2, ADD)

                xT_fl = psxT.tile([128, TILE], f32r, name=f"xT_{t}", tag="xt")
                xT_ps = xT_fl[0:10, :]
                for g in range(GROUPS):
                    nc.tensor.transpose(
                        xT_ps[:, g * 128:(g + 1) * 128], x_nat[:, g, :],
                        ident_r)
                xT = acts.tile([10, TILE], f32r, name=f"xTs_{t}", tag="xT_sb")
                nc.vector.tensor_copy(xT[:, 0:half], xT_ps[:, 0:half])
                nc.scalar.activation(xT[:, half:TILE], xT_ps[:, half:TILE],
                                     IDENT)
                return z_nat, xT

            def emit_late(t, z_nat, stk_sb):
                """T transpose + evac, exp, polynomial trig, rotation."""
                z1 = z_nat[:, :, 4:16:2]
                z2 = z_nat[:, :, 5:16:2]

                T_fl = psxT.tile([128, TILE], f32r, name=f"T_{t}", tag="xt")
                for g in range(GROUPS):
                    nc.tensor.transpose(
                        T_fl[:, g * 16:(g + 1) * 16],
                        stk_sb[:, g * 128:(g + 1) * 128],
                        ident_r[0:16, 0:16])
                T_sb = epi.tile([128, GROUPS, 16], f32, name=f"Ts_{t}",
                                tag="T_sb")
                nc.vector.tensor_copy(T_sb, T_fl[:, 0:GROUPS * 16])

                lamT = T_sb[:, :, 0:4]
                muT = T_sb[:, :, 4:10]
                omT = T_sb[:, :, 10:16]

                # e = exp(mu) via degree-6 Horner on Pool (keeps ACT free
                # for the relu passes)
                e = epi.tile([128, GROUPS, 6], f32, name=f"e_{t}", tag="e")
                P.tensor_tensor(e, muT, kexp[6], MULT)
                P.tensor_tensor(e, e, kexp[5], ADD)
                for k in range(4, -1, -1):
                    P.tensor_tensor(e, e, muT, MULT)
                    P.tensor_tensor(e, e, kexp[k], ADD)

                # cos(om)*e ~ e + r*e       with r = (w2/24 - 1/2)*w2
                # sin(om)*e ~ (p*om + om)*e with p = (w2/120 - 1/6)*w2
                w2 = epi.tile([128, GROUPS, 6], f32, name=f"w2_{t}", tag="w2")
                P.tensor_tensor(w2, omT, omT, MULT)
                tc_ = epi.tile([128, GROUPS, 6], f32, name=f"tc_{t}", tag="tc")
                P.tensor_tensor(tc_, w2, k24, MULT)
                P.tensor_tensor(tc_, tc_, kmh, ADD)
                rc = epi.tile([128, GROUPS, 6], f32, name=f"rc_{t}", tag="rc")
                P.tensor_tensor(rc, tc_, w2, MULT)
                mc = epi.tile([128, GROUPS, 6], f32, name=f"mc_{t}", tag="mc")
                P.tensor_tensor(mc, rc, e, MULT)
                P.tensor_tensor(mc, mc, e, ADD)
                ts_ = epi.tile([128, GROUPS, 6], f32, name=f"ts_{t}", tag="ts")
                P.tensor_tensor(ts_, w2, k120, MULT)
                P.tensor_tensor(ts_, ts_, km6, ADD)
                rs = epi.tile([128, GROUPS, 6], f32, name=f"rs_{t}", tag="rs")
                P.tensor_tensor(rs, ts_, w2, MULT)
                sn = epi.tile([128, GROUPS, 6], f32, name=f"sn_{t}", tag="sn")
                P.tensor_tensor(sn, rs, omT, MULT)
                P.tensor_tensor(sn, sn, omT, ADD)
                ms = epi.tile([128, GROUPS, 6], f32, name=f"ms_{t}", tag="ms")
                P.tensor_tensor(ms, sn, e, MULT)

                # out_r = zr*lam; o1 = z1*mc + z2*ms; o2 = z2*mc - z1*ms
                o_nat = io.tile([128, GROUPS, C], f32, name=f"o_nat_{t}",
                                tag="o_nat")
                P.tensor_tensor(o_nat[:, :, 0:4], z_nat[:, :, 0:4], lamT,
                                MULT)
                t1 = epi.tile([128, GROUPS, 6], f32, name=f"t1_{t}", tag="t1")
                t2 = epi.tile([128, GROUPS, 6], f32, name=f"t2_{t}", tag="t2")
                P.tensor_tensor(t1, z1, mc, MULT)
                P.tensor_tensor(t2, z2, ms, MULT)
                P.tensor_tensor(o_nat[:, :, 4:16:2], t1, t2, ADD)
                t3 = epi.tile([128, GROUPS, 6], f32, name=f"t3_{t}", tag="t3")
                t4 = epi.tile([128, GROUPS, 6], f32, name=f"t4_{t}", tag="t4")
                P.tensor_tensor(t3, z2, mc, MULT)
                P.tensor_tensor(t4, z1, ms, MULT)
                P.tensor_tensor(o_nat[:, :, 5:16:2], t3, t4, SUB)

                nc.sync.dma_start(out=out_r[t], in_=o_nat)

            # --- software-pipelined main loop ---
            late_args = None      # tile t-1 epilogue inputs
            cur = emit_prelude(0)
            for t in range(NT):
                z_nat, xT = cur
                rhs = [xT] * 5
                pss = [None] * 5

                for s in range(3):
                    weights = w0_sb if s == 0 else wm_sb[s - 1]
                    for j in range(5):
                        pss[j] = pshid.tile([128, TILE], f32,
                                            name=f"ps_{t}_{s}_{j}", tag="ps")
                        nc.tensor.matmul(pss[j], weights[j], rhs[j],
                                         start=True, stop=True)
                    if s == 0 and t + 1 < NT:
                        # next tile's prelude: PE transposes + Pool x-prep
                        # fill the gap before this tile's stage-1 matmuls
                        cur = emit_prelude(t + 1)
                    if s == 1 and late_args is not None:
                        # previous tile's epilogue fills the stage-2 gap
                        emit_late(*late_args)
                        late_args = None
                    for j in range(5):
                        h = acts.tile([128, TILE], f32r,
                                      name=f"h_{t}_{s}_{j}", tag="h")
                        relu0(RELU_ASSIGN[j][s], h, pss[j])
                        rhs[j] = h

                # stk shares the pshid ring slot rotation
                stk_fl = pshid.tile([128, TILE], f32, name=f"stk_{t}",
                                    tag="ps")
                stk_ps = stk_fl[0:16, :]
                for j in range(5):
                    nc.tensor.matmul(stk_ps, wl_sb[j], rhs[j],
                                     start=(j == 0), stop=(j == 4))

                # evacuate stk to SBUF, split across DVE and ACT
                # (biases are all zero per the spec)
                stk_sb = acts.tile([16, TILE], f32r, name=f"stks_{t}",
                                   tag="stk_sb")
                nc.vector.tensor_copy(stk_sb[:, 0:half], stk_ps[:, 0:half])
                nc.scalar.activation(stk_sb[:, half:TILE],
                                     stk_ps[:, half:TILE], IDENT)

                late_args = (t, z_nat, stk_sb)

            emit_late(*late_args)

    nc.compile()
    return nc


def _pack_weights(i):
    """Pack per-channel weights into block-diagonal pair form.

    All biases must be zero (guaranteed by the problem spec, fill=zeros);
    the kernel folds that assumption into pure-relu passes.
    """
    f32 = np.float32
    for k in ("b0_r", "bm_r", "bl_r", "b0_c", "bm_c", "bl_c"):
        assert not np.any(np.asarray(i[k])), f"nonzero bias {k}"
    W0_r = np.asarray(i["W0_r"], f32)
    Wm_r = np.asarray(i["Wm_r"], f32)
    Wl_r = np.asarray(i["Wl_r"], f32)
    W0_c = np.asarray(i["W0_c"], f32)
    Wm_c = np.asarray(i["Wm_c"], f32)
    Wl_c = np.asarray(i["Wl_c"], f32)

    w0p = np.zeros((5, 10, 128), f32)
    wmp = np.zeros((L, 5, 128, 128), f32)
    wlp = np.zeros((5, 128, 16), f32)
    for j in range(5):
        if j < 2:
            a, b = 2 * j, 2 * j + 1
            W0, Wm = W0_r, Wm_r
        else:
            a, b = 2 * (j - 2), 2 * (j - 2) + 1
            W0, Wm = W0_c, Wm_c
        r0 = 2 * j if j < 2 else 4 + 2 * (j - 2)
        w0p[j, r0, 0:64] = W0[a]
        w0p[j, r0 + 1, 64:128] = W0[b]
        for l in range(L):
            wmp[l, j, 0:64, 0:64] = Wm[l, a]
            wmp[l, j, 64:128, 64:128] = Wm[l, b]
        if j < 2:
            wlp[j, 0:64, 2 * j] = Wl_r[a][:, 0]
            wlp[j, 64:128, 2 * j + 1] = Wl_r[b][:, 0]
        else:
            jc = j - 2
            wlp[j, 0:64, 4 + 2 * jc] = Wl_c[a][:, 0]        # mu_a
            wlp[j, 64:128, 5 + 2 * jc] = Wl_c[b][:, 0]      # mu_b
            wlp[j, 0:64, 10 + 2 * jc] = Wl_c[a][:, 1]       # om_a
            wlp[j, 64:128, 11 + 2 * jc] = Wl_c[b][:, 1]     # om_b

    w0_all = np.concatenate([w0p[j] for j in range(5)], axis=1)      # [10,640]
    wm_all = np.concatenate(
        [wmp[l, j] for l in range(L) for j in range(5)], axis=1)  # [128,1280]
    wl_all = np.concatenate([wlp[j] for j in range(5)], axis=1)      # [128,80]
    return {"w0p": np.ascontiguousarray(w0_all),
            "wmp": np.ascontiguousarray(wm_all),
            "wlp": np.ascontiguousarray(wl_all)}


def kernel(**inputs):
    global _cached_nc
    if _cached_nc is None:
        _cached_nc = _build()
    nc = _cached_nc

    from concourse.bass_utils import run_bass_kernel_spmd

    weights = _pack_weights(inputs)
    z = np.ascontiguousarray(np.asarray(inputs["z"], np.float32)
                             .reshape(NCORES, F_CORE, C))
    in_maps = [dict(weights, z=z[i]) for i in range(NCORES)]
    res = run_bass_kernel_spmd(nc, in_maps, core_ids=list(range(NCORES)))
    outs = [np.asarray(res.results[i]["out"]) for i in range(NCORES)]
    return np.concatenate(outs, axis=0).reshape(B, S, C)
